# revision 37
# baseline (speedup 1.0000x reference)
"""Trainium2 Bass kernel for DeepKernelNN GNN message passing (NNConv-style).

Strategy (8 NeuronCores, SPMD):
  - Host: sort edges by dst, shard contiguous 512-node dst ranges per core,
    pad each core to a common edge count. Precompute h0 = x@fc1+b (tiny),
    per-edge metadata (src idx, local dst, 1/deg), and augmented weights.
  - Device per layer: edge MLP feature-major (weights stationary on PE,
    fp32r 1 cyc/row), We = e2@kw3 edge-major via activation-stationary
    matmuls into PSUM, per-edge matvec msg = h[src] . We on DVE with
    per-partition-scalar FMAs, segment-sum via one-hot scatter matmul
    (S built on device from iota/is_equal, inv_deg folded in), NNConv
    update feature-major, AllGather h across the 8 cores per layer.
"""

import sys

sys.path.insert(0, "/opt/trn_rl_repo")

import numpy as np

import concourse.bass as bass
import concourse.mybir as mybir
import concourse.tile as tile_mod
from concourse.bass_utils import run_bass_kernel_spmd
from concourse.masks import make_identity
from concourse.tile import TileContext
from concourse.vector_clock import ScopedClock, VectorClock

# ----------------------------------------------------------------------------
# Toolchain workarounds: this walrus build rejects instructions carrying more
# than a couple of sync waits ("Too many sync wait commands").  Split waits
# onto dedicated same-engine NoOps.
# ----------------------------------------------------------------------------
WAIT_LIMIT = 1


def _patched_drain_and_barrier(self, tick_clock, wait_clock):
    nc = self.nc
    gc = tick_clock.global_clock
    n = len(gc)
    for i in range(n):
        t = gc[i]
        if t > 0:
            sub = [0] * n
            sub[i] = t
            nop_inst = nc.sync.nop(nofuse=True)
            wait_clock.add_sem_waits(nop_inst.ins, ScopedClock({None: VectorClock(sub)}))
    nc.sync.drain()
    nc.all_engine_barrier()
    popped = nc._tile_sem_poison_stack.pop()
    assert popped is self._sem_poison
    nc.clear_and_free_semaphores(list(self.sems.allocated().values()))
    nc.all_engine_barrier()


tile_mod.TileContext._drain_and_barrier = _patched_drain_and_barrier


def _split_excess_waits(nc, limit=WAIT_LIMIT):
    n_split = 0
    for _bbname, bbb in nc.bb_map.items():
        bb = bbb.bb
        insts = list(bb.instructions)
        out = []
        for inst in insts:
            si = inst.sync_info
            if si is not None and si.on_wait is not None and len(si.on_wait) > limit:
                waits = list(si.on_wait)
                movable = [w for w in waits if w.wait_reg is None]
                fixed = [w for w in waits if w.wait_reg is not None]
                nkeep_mov = max(0, limit - len(fixed))
                keep = fixed + (movable[len(movable) - nkeep_mov:] if nkeep_mov else [])
                extra = movable[: len(movable) - nkeep_mov]
                while extra:
                    chunk, extra = extra[:limit], extra[limit:]
                    nop = mybir.InstNoOp(name=nc.get_next_instruction_name())
                    nop.engine = inst.engine
                    nop.sync_info = mybir.SyncInfo(on_wait=chunk, on_update=[])
                    nop.bass_nofuse = True
                    nc.register_instruction(nop, overwrite=True)
                    out.append(nop)
                    n_split += 1
                inst.sync_info = mybir.SyncInfo(
                    on_wait=keep, on_update=list(si.on_update or [])
                )
            out.append(inst)
        bb.instructions = out
    return n_split


import concourse.bass_utils as _bu

_orig_run_command = _bu.run_command


def _run_command_no_birverifier(cmd, **kw):
    cmd = [
        c.replace("birverifier,", "") if isinstance(c, str) else c for c in cmd
    ]
    return _orig_run_command(cmd, **kw)


_bu.run_command = _run_command_no_birverifier


def _round_f32r(x):
    """Host replica of the fp32r rounding (RNE, 11 mantissa bits kept).

    PE fp32r operands must contain rounded bits: feeding raw fp32 bits makes
    the PE fault (verified on HW), so anything DMA'd into an fp32r tile is
    pre-rounded here.
    """
    ai = np.ascontiguousarray(x, np.float32).view(np.uint32)
    drop = np.uint32(12)
    half = np.uint32(1 << 11)
    lsb = ((ai >> drop) & np.uint32(1)).astype(np.uint32)
    out = ((ai + (half - np.uint32(1)) + lsb) >> drop) << drop
    return out.view(np.float32)


# ----------------------------------------------------------------------------
# Problem constants (hardcoded from the model definition)
# ----------------------------------------------------------------------------
N_NODES = 4096
N_EDGES = 32768
WIDTH = 64
KER_W = 1024
DEPTH = 4
KER_IN = 6
IN_W = 6
NCORES = 8
NLOC = N_NODES // NCORES  # 512 nodes per core
P = 128

_dt = mybir.dt
F32 = _dt.float32
F32R = _dt.float32r
I32 = _dt.int32
ALU = mybir.AluOpType
AF = mybir.ActivationFunctionType
import os as _os
NACCS = int(_os.environ.get("MATVEC_ACCS", "2"))
ABLATE = _os.environ.get("ABLATE", "")


def _build_nc(T, kb3_nonzero):
    """Build the SPMD Bass program for T 128-edge tiles per core."""
    B = (T + 3) // 4  # blocks of 512 edges (last may be ragged)
    EP = B * 512
    nc = bass.Bass("TRN2", target_bir_lowering=False, debug=False, num_devices=NCORES)

    # ---- dram inputs (per-core in_maps supply the data) ----
    d_h0g = nc.dram_tensor("h0g", [N_NODES, WIDTH], F32, kind="ExternalInput")
    d_hfm0 = nc.dram_tensor("hfm0", [WIDTH, NLOC], F32, kind="ExternalInput")
    d_kw1 = nc.dram_tensor("kw1a", [DEPTH, IN_W + 1, KER_W // 2], F32, kind="ExternalInput")
    d_kw2 = nc.dram_tensor("kw2", [DEPTH, KER_W // 2, KER_W], F32, kind="ExternalInput")
    d_kb2 = nc.dram_tensor("kb2s", [DEPTH, P, KER_W // P], F32, kind="ExternalInput")
    d_kw3 = nc.dram_tensor("kw3", [DEPTH, KER_W, WIDTH * WIDTH], F32, kind="ExternalInput")
    d_root = nc.dram_tensor("roota", [DEPTH, WIDTH + 1, WIDTH], F32, kind="ExternalInput")
    d_fc2 = nc.dram_tensor("fc2a", [WIDTH + 1, 1], F32, kind="ExternalInput")
    d_ea = nc.dram_tensor("eaT", [IN_W + 1, EP], F32, kind="ExternalInput")
    d_src = nc.dram_tensor("srci", [EP, 1], I32, kind="ExternalInput")
    d_dst = nc.dram_tensor("dstl", [EP, 1], F32, kind="ExternalInput")
    d_inv = nc.dram_tensor("invde", [EP, 1], F32, kind="ExternalInput")
    d_iota = nc.dram_tensor("iota", [P, NLOC], F32, kind="ExternalInput")
    d_kb3 = None
    if kb3_nonzero:
        d_kb3 = nc.dram_tensor("kb3r", [DEPTH, WIDTH, WIDTH], F32, kind="ExternalInput")

    d_out = nc.dram_tensor("out_loc", [1, NLOC], F32, kind="ExternalOutput")

    # ---- internal dram ----
    d_hloc = nc.dram_tensor("hloc", [NLOC, WIDTH], F32)
    d_hgat = [
        nc.dram_tensor(f"hgat{k}", [N_NODES, WIDTH], F32, addr_space="Shared")
        for k in range(DEPTH - 1)
    ]

    rg = [list(range(NCORES))]
    KC3 = KER_W // P  # 8 contraction chunks for kw3
    NC3 = WIDTH * WIDTH  # 4096 output cols

    with TileContext(nc) as tc:
        with (
            tc.tile_pool(name="pers", bufs=1) as pers,
            tc.tile_pool(name="wk", bufs=2) as wk,
            tc.tile_pool(name="ppw", bufs=2, space="PSUM") as ppw,
            tc.tile_pool(name="ppe", bufs=2, space="PSUM") as ppe,
            tc.tile_pool(name="ppm", bufs=2, space="PSUM") as ppm,
            tc.tile_pool(name="ppa", bufs=1, space="PSUM") as ppa,
        ):
            # ---------------- persistent tiles ----------------
            iota_s = pers.tile([P, NLOC], F32)
            nc.sync.dma_start(out=iota_s[:], in_=d_iota[:])
            ident = pers.tile([P, P], F32)
            make_identity(nc, ident[:])

            # per-edge metadata as [128, 4B] (covers T used tiles)
            srcT = pers.tile([P, 4 * B], I32)
            dstT = pers.tile([P, 4 * B], F32)
            invT = pers.tile([P, 4 * B], F32)
            for (dsttile, dram) in ((srcT, d_src), (dstT, d_dst), (invT, d_inv)):
                nc.sync.dma_start(
                    out=dsttile[:],
                    in_=dram.ap().rearrange("(t p) o -> p (t o)", p=P),
                )



            # h feature-major augmented [65, 512]; row 64 = ones
            # (hfm0 pre-rounded on host; device relu copies re-round later)
            hfm = pers.tile([WIDTH + 1, NLOC], F32R)
            nc.sync.dma_start(out=hfm[0:WIDTH, :].bitcast(F32), in_=d_hfm0[:])
            nc.vector.memset(hfm[WIDTH : WIDTH + 1, :].bitcast(F32), 1.0)

            fc2r = pers.tile([WIDTH + 1, 1], F32R)
            nc.sync.dma_start(out=fc2r[:].bitcast(F32), in_=d_fc2[:])

            # per-layer weight tiles (persistent slots, reloaded per layer;
            # per-chunk tiles so reloads only WAR-wait on their own chunk)
            kw1r = pers.tile([IN_W + 1, KER_W // 2], F32R)
            kw2rc = [
                pers.tile([P, KER_W], F32R, name=f"kw2rc{c}")
                for c in range(KER_W // 2 // P)
            ]
            kw3rc = [
                pers.tile([P, NC3], F32R, name=f"kw3rc{c}") for c in range(KC3)
            ]
            rootr = pers.tile([WIDTH + 1, WIDTH], F32R)
            kb2t = pers.tile([P, KER_W // P], F32)
            kb3t = pers.tile([WIDTH, WIDTH], F32R) if kb3_nonzero else None

            # working tiles allocated per use from `wk`
            e1r = pers.tile([P, 4 * 512], F32R)

            for k in range(DEPTH):
                # ------------ load layer weights (pre-rounded on host) ------------
                if ABLATE != "reuseweights" or k == 0:
                    nc.sync.dma_start(out=kw1r[:].bitcast(F32), in_=d_kw1[k])
                    for c in range(KER_W // 2 // P):
                        nc.sync.dma_start(
                            out=kw2rc[c][:].bitcast(F32), in_=d_kw2[k, c * P : (c + 1) * P, :]
                        )
                    for kc in range(KC3):
                        nc.sync.dma_start(
                            out=kw3rc[kc][:].bitcast(F32), in_=d_kw3[k, kc * P : (kc + 1) * P, :]
                        )
                    nc.sync.dma_start(out=rootr[:].bitcast(F32), in_=d_root[k])
                    nc.sync.dma_start(out=kb2t[:], in_=d_kb2[k])
                if kb3_nonzero:
                    nc.sync.dma_start(out=kb3t[:].bitcast(F32), in_=d_kb3[k])

                aggP = ppa.tile([WIDTH, NLOC], F32, tag="aggP")
                htab = d_h0g if k == 0 else d_hgat[k - 1]

                for blk in range(B):
                    eoff = blk * 512
                    e2r = wk.tile([P, 8 * 512], F32R, tag="e2r")
                    # ---- this block's edge attrs [7, 512] (host pre-rounded) ----
                    ea_rb = wk.tile([IN_W + 1, 512], F32R, tag="ea_rb")
                    nc.sync.dma_start(out=ea_rb[:].bitcast(F32), in_=d_ea[:, eoff : eoff + 512])
                    # ---- e1 = relu(ea @ kw1_aug) : [512 feats, 512 edges] ----
                    for mc in range(4):
                        pe1 = ppe.tile([P, 512], F32, tag="pe")
                        nc.tensor.matmul(
                            out=pe1[:],
                            lhsT=kw1r[:, mc * P : (mc + 1) * P],
                            rhs=ea_rb[:],
                            start=True,
                            stop=True,
                        )
                        nc.scalar.activation(
                            e1r[:, mc * 512 : (mc + 1) * 512], pe1[:], AF.Relu
                        )
                    # ---- e2 = relu(e1 @ kw2 + kb2) : [1024 feats, 512 edges] ----
                    for mc2 in range(8):
                        pe2 = ppe.tile([P, 512], F32, tag="pe")
                        for kc in range(4):
                            nc.tensor.matmul(
                                out=pe2[:],
                                lhsT=kw2rc[kc][:, mc2 * P : (mc2 + 1) * P],
                                rhs=e1r[:, kc * 512 : (kc + 1) * 512],
                                start=(kc == 0),
                                stop=(kc == 3),
                            )
                        nc.scalar.activation(
                            e2r[:, mc2 * 512 : (mc2 + 1) * 512],
                            pe2[:],
                            AF.Relu,
                            bias=kb2t[:, mc2 : mc2 + 1],
                        )
                    # ---- per 128-edge tile (ragged last block) ----
                    for t4 in range(min(4, T - blk * 4)):
                        t = blk * 4 + t4
                        hsrc = wk.tile([P, WIDTH], F32, tag="hsrc")
                        if ABLATE != "nogather":
                            nc.gpsimd.indirect_dma_start(
                                out=hsrc[:],
                                out_offset=None,
                                in_=htab[:],
                                in_offset=bass.IndirectOffsetOnAxis(
                                    ap=srcT[:, t : t + 1], axis=0
                                ),
                            )
                        else:
                            nc.vector.memset(hsrc[:], 0.01)
                        # 8 independent accumulators keep the DVE pipeline full
                        # (serial FMA chain measured 286 ns/op vs 88 ns with 8)
                        accs = [
                            wk.tile([P, WIDTH], F32, name=f"macc{j}_{t}", tag=f"macc{j}", bufs=2)
                            for j in range(NACCS)
                        ]
                        msgr = wk.tile([P, WIDTH], F32R, tag="msgr")
                        tcor = None
                        if kb3_nonzero:
                            tps = ppm.tile([WIDTH, P], F32, tag="tp")
                            nc.tensor.transpose(out=tps[:], in_=hsrc[:], identity=ident[:])
                            hsT = wk.tile([WIDTH, P], F32R, tag="hsT")
                            nc.scalar.activation(hsT[:], tps[:], AF.Copy)
                            tcor = ppm.tile([P, WIDTH], F32, tag="tc")
                            nc.tensor.matmul(
                                out=tcor[:], lhsT=hsT[:], rhs=kb3t[:], start=True, stop=True
                            )
                        for cc in range(8):  # 512-col chunks of We
                            wps = ppw.tile([P, 512], F32, tag="wps")
                            for kc in range(KC3):
                                nc.tensor.matmul(
                                    out=wps[:],
                                    lhsT=e2r[:, kc * 512 + t4 * P : kc * 512 + (t4 + 1) * P],
                                    rhs=kw3rc[kc][:, cc * 512 : (cc + 1) * 512],
                                    start=(kc == 0),
                                    stop=(kc == KC3 - 1),
                                )
                            for j in range((0 if ABLATE == "nomatvec" else 8)):
                                i_ = cc * 8 + j
                                sl = wps[:, j * WIDTH : (j + 1) * WIDTH]
                                sc = hsrc[:, i_ : i_ + 1]
                                ja = j % NACCS
                                if cc * 8 + j < NACCS:
                                    nc.vector.tensor_scalar(
                                        out=accs[ja][:], in0=sl, scalar1=sc,
                                        scalar2=None, op0=ALU.mult,
                                    )
                                else:
                                    nc.vector.scalar_tensor_tensor(
                                        out=accs[ja][:], in0=sl, scalar=sc,
                                        in1=accs[ja][:], op0=ALU.mult, op1=ALU.add,
                                    )
                        if ABLATE == "nomatvec":
                            for j in range(NACCS):
                                nc.vector.memset(accs[j][:], 0.0)
                        # tree-reduce the accumulators
                        stride = 1
                        while stride < NACCS:
                            for d in range(0, NACCS, 2 * stride):
                                if d + stride < NACCS and not (
                                    stride * 2 >= NACCS and d == 0 and not kb3_nonzero
                                ):
                                    nc.vector.tensor_add(
                                        out=accs[d][:], in0=accs[d][:], in1=accs[d + stride][:]
                                    )
                            stride *= 2
                        if kb3_nonzero:
                            nc.vector.tensor_add(out=accs[0][:], in0=accs[0][:], in1=tcor[:])
                            nc.vector.tensor_copy(out=msgr[:], in_=accs[0][:])
                        elif NACCS > 1:
                            nc.vector.tensor_add(
                                out=msgr[:], in0=accs[0][:], in1=accs[NACCS // 2][:]
                            )
                        else:
                            nc.vector.tensor_copy(out=msgr[:], in_=accs[0][:])
                        # ---- scatter: one-hot (iota==dst)*invdeg, feature-major agg ----
                        S = wk.tile([P, NLOC], F32R, tag="S")
                        if ABLATE != "noscatter":
                            nc.vector.tensor_scalar(
                                out=S[:], in0=iota_s[:], scalar1=dstT[:, t : t + 1],
                                scalar2=invT[:, t : t + 1], op0=ALU.is_equal, op1=ALU.mult,
                            )
                            nc.tensor.matmul(
                                out=aggP[:], lhsT=msgr[:], rhs=S[:],
                                start=(t == 0), stop=False, skip_group_check=True,
                            )
                        elif t == 0:
                            nc.vector.memset(S[:].bitcast(F32), 0.0)
                            nc.tensor.matmul(
                                out=aggP[:], lhsT=msgr[:], rhs=S[:],
                                start=True, stop=False, skip_group_check=True,
                            )
                # ---- update: h = relu(agg*inv_deg(folded) + h@root + bias) ----
                nc.tensor.matmul(
                    out=aggP[:], lhsT=rootr[:], rhs=hfm[:],
                    start=False, stop=True, skip_group_check=True,
                )
                hnf = wk.tile([WIDTH, NLOC], F32, tag="hnf")
                nc.scalar.activation(hnf[:], aggP[:], AF.Relu)
                nc.scalar.activation(hfm[0:WIDTH, :], hnf[:], AF.Copy)
                if k < DEPTH - 1:
                    for c in range(NLOC // P):
                        tp = ppm.tile([P, WIDTH], F32, tag="tp")
                        nc.tensor.transpose(
                            out=tp[:],
                            in_=hnf[:, c * P : (c + 1) * P],
                            identity=ident[0:WIDTH, 0:WIDTH],
                        )
                        hts = wk.tile([P, WIDTH], F32, tag="hts")
                        nc.vector.tensor_copy(out=hts[:], in_=tp[:])
                        nc.sync.dma_start(out=d_hloc[c * P : (c + 1) * P, :], in_=hts[:])
                    nc.gpsimd.collective_compute(
                        "AllGather",
                        ALU.bypass,
                        ins=[d_hloc[:]],
                        outs=[d_hgat[k][:]],
                        replica_groups=rg,
                    )
            # ---- readout: out = h @ fc2 + b ----
            pf = ppm.tile([1, NLOC], F32, tag="tp")
            nc.tensor.matmul(out=pf[:], lhsT=fc2r[:], rhs=hfm[:], start=True, stop=True)
            ot = wk.tile([1, NLOC], F32, tag="hnf")
            nc.vector.tensor_copy(out=ot[:], in_=pf[:])
            nc.sync.dma_start(out=d_out[:], in_=ot[:])

    _split_excess_waits(nc)
    return nc


def _host_prep(x, edge_attr, fc1_w, fc1_b, kw1, kb1, kw2, kb2, kw3, kb3,
               root, bias, fc2_w, fc2_b, edge_index):
    f = np.float32
    x = np.asarray(x, f)
    edge_attr = np.asarray(edge_attr, f)
    fc1_w = np.asarray(fc1_w, f); fc1_b = np.asarray(fc1_b, f)
    kw1 = np.asarray(kw1, f); kb1 = np.asarray(kb1, f)
    kw2 = np.asarray(kw2, f); kb2 = np.asarray(kb2, f)
    kw3 = np.asarray(kw3, f); kb3 = np.asarray(kb3, f)
    root = np.asarray(root, f); bias = np.asarray(bias, f)
    fc2_w = np.asarray(fc2_w, f); fc2_b = np.asarray(fc2_b, f)
    ei = np.asarray(edge_index)
    src = ei[0].astype(np.int64)
    dst = ei[1].astype(np.int64)

    deg = np.bincount(dst, minlength=N_NODES).astype(f)
    inv_deg = np.zeros(N_NODES, f)
    np.divide(f(1.0), deg, out=inv_deg, where=deg > 0)

    order = np.argsort(dst, kind="stable")
    dsts = dst[order]
    bounds = np.searchsorted(dsts, np.arange(0, N_NODES + 1, NLOC))
    counts = np.diff(bounds)
    T = int(np.ceil(counts.max() / 128.0))
    EP = ((T + 3) // 4) * 512

    h0 = (x @ fc1_w + fc1_b).astype(f)

    kw1_aug = _round_f32r(np.concatenate([kw1, kb1[:, None, :]], axis=1))
    kw2 = _round_f32r(kw2)
    kw3 = _round_f32r(kw3)
    kb2s = np.stack([kb2[k].reshape(KER_W // P, P).T for k in range(DEPTH)]).astype(f)
    root_aug = _round_f32r(np.concatenate([root, bias[:, None, :]], axis=1))
    fc2_aug = _round_f32r(np.concatenate([fc2_w, fc2_b.reshape(1, 1)], axis=0))
    iota = np.tile(np.arange(NLOC, dtype=f)[None, :], (P, 1))
    kb3_nonzero = bool(np.any(kb3))
    kb3r = _round_f32r(kb3.reshape(DEPTH, WIDTH, WIDTH))

    in_maps = []
    for m in range(NCORES):
        sel = order[bounds[m] : bounds[m + 1]]
        cnt = len(sel)
        eaT = np.zeros((IN_W + 1, EP), f)
        eaT[0:IN_W, :cnt] = edge_attr[sel].T
        eaT[IN_W, :cnt] = 1.0
        eaT = _round_f32r(eaT)
        srci = np.zeros((EP, 1), np.int32)
        srci[:cnt, 0] = src[sel].astype(np.int32)
        dstl = np.full((EP, 1), -1.0, f)
        dstl[:cnt, 0] = (dst[sel] - NLOC * m).astype(f)
        invde = np.zeros((EP, 1), f)
        invde[:cnt, 0] = inv_deg[dst[sel]]
        im = {
            "h0g": h0,
            "hfm0": _round_f32r(np.ascontiguousarray(h0[NLOC * m : NLOC * (m + 1)].T)),
            "kw1a": kw1_aug,
            "kw2": kw2,
            "kb2s": kb2s,
            "kw3": kw3,
            "roota": root_aug,
            "fc2a": fc2_aug,
            "eaT": eaT,
            "srci": srci,
            "dstl": dstl,
            "invde": invde,
            "iota": iota,
        }
        if kb3_nonzero:
            im["kb3r"] = kb3r
        in_maps.append(im)
    return in_maps, T, kb3_nonzero


_BUILD_CACHE = {}


def kernel(**inputs) -> np.ndarray:
    in_maps, T, kb3_nonzero = _host_prep(**inputs)
    key = (T, kb3_nonzero)
    if key not in _BUILD_CACHE:
        _BUILD_CACHE[key] = _build_nc(T, kb3_nonzero)
    nc = _BUILD_CACHE[key]
    res = run_bass_kernel_spmd(nc, in_maps, list(range(NCORES)))
    out = np.concatenate(
        [res.results[m]["out_loc"].reshape(NLOC, 1) for m in range(NCORES)], axis=0
    )
    return out.astype(np.float32)
